# revision 10
# baseline (speedup 1.0000x reference)
"""Gemma4 vision pooler (position-indexed 4x4 average pool) on 8 TRN2 cores.

Strategy: pure data parallel — batch element b -> core b. On each core the
pooling is a segment reduce over 4096 rows into 256 segments of 16 rows,
done as one-hot matmuls on the tensor engine:

    out[l, h] = sum_s onehot(kidx[s] == l) * hs[s, h],  then * sqrt(H)/16

The host precomputes kidx (the segment id per row, exactly mirroring the
reference index math), splits hs into bf16 hi+lo halves (x ~= hi + lo with
~2^-18 relative error, so the bf16 tensor engine path is near-fp32 exact at
1 cycle/row instead of fp32's 4), pre-transposes to a [128, 32*2304] layout
so every DMA is contiguous per partition, and builds the one-hot masks ON
DEVICE from kidx via iota + is_equal (the 4 MB one-hot never crosses HBM).
"""

import numpy as np

P = 128          # partitions
H = 1152         # hidden size
S = 4096         # sequence length
L = 256          # output length
NT = S // P      # 32 s-tiles of 128 rows
W = 2 * H        # bf16 hi|lo row width per s-tile
TPC = 2          # s-tiles per DMA chunk
NCHUNK = NT // TPC
NHC = 3          # h chunks per matmul group
HC = H // NHC    # 384
N_CORES = 8
TILES_PER_LC = NT // 2  # 16 s-tiles accumulate into each 128-row output chunk

TRACE = False          # set by test harness to capture an NTFF profile
LAST_EXEC_NS = None    # filled when TRACE is set
LAST_RESULTS = None

_compiled_nc = None


def _build_nc():
    from contextlib import ExitStack

    import concourse.bacc as bacc
    import concourse.tile as tile
    from concourse import mybir

    nc = bacc.Bacc("TRN2", target_bir_lowering=False, debug=False)

    # per s-tile t, cols [t*W, t*W+H) = bf16 hi, [t*W+H, (t+1)*W) = bf16 lo
    hsT = nc.dram_tensor("hsT", [P, NT * W], mybir.dt.bfloat16, kind="ExternalInput")
    kidxT = nc.dram_tensor("kidxT", [P, NT], mybir.dt.int32, kind="ExternalInput")
    out = nc.dram_tensor("out", [L, H], mybir.dt.float32, kind="ExternalOutput")

    scale = float(np.float32(np.sqrt(np.float64(H)) / 16.0))

    with ExitStack() as ctx:
        tc = ctx.enter_context(tile.TileContext(nc))
        const_pool = ctx.enter_context(tc.tile_pool(name="const", bufs=1))
        hs_pool = ctx.enter_context(tc.tile_pool(name="hs", bufs=NCHUNK))
        mask_pool = ctx.enter_context(tc.tile_pool(name="mask", bufs=NT))
        out_pool = ctx.enter_context(tc.tile_pool(name="outp", bufs=2))
        psum_pool = ctx.enter_context(tc.tile_pool(name="psum", bufs=1, space="PSUM"))

        kidx_i = const_pool.tile([P, NT], mybir.dt.int32, tag="kidx_i")
        nc.scalar.dma_start(kidx_i[:], kidxT[:])
        kidx_f = const_pool.tile([P, NT], mybir.dt.float32, tag="kidx_f")
        nc.vector.tensor_copy(kidx_f[:], kidx_i[:])

        iotas = []
        for lc in range(2):
            it = const_pool.tile([P, P], mybir.dt.int32, tag=f"iota_i{lc}")
            nc.gpsimd.iota(it[:], pattern=[[1, P]], base=lc * P, channel_multiplier=0)
            itf = const_pool.tile([P, P], mybir.dt.float32, tag=f"iota_f{lc}")
            nc.vector.tensor_copy(itf[:], it[:])
            iotas.append(itf)

        # chunk layout in s-tiles: 15 chunks of 2 tiles + 2 tail chunks of 1
        # tile (shorter final matmul burst after the last DMA lands);
        # alternate the two HWDGE rings (sync=SP, scalar=Act) for the input.
        chunk_sizes = [2] * 15 + [1, 1]
        assert sum(chunk_sizes) == NT
        tile_to_chunk = {}
        chunk_off = {}
        chunks = []
        t0 = 0
        for c, sz in enumerate(chunk_sizes):
            ch = hs_pool.tile([P, sz * W], mybir.dt.bfloat16, tag="ch", name=f"ch{c}")
            eng = nc.sync if c % 2 == 0 else nc.scalar
            eng.dma_start(ch[:], hsT[:, t0 * W : (t0 + sz) * W])
            chunks.append(ch)
            for j in range(sz):
                tile_to_chunk[t0 + j] = c
                chunk_off[t0 + j] = j * W
            t0 += sz

        for lc in range(2):
            ps = [
                psum_pool.tile([P, HC], mybir.dt.float32, tag=f"ps{lc}_{hc}", name=f"ps{lc}_{hc}")
                for hc in range(NHC)
            ]
            for j in range(TILES_PER_LC):
                t = lc * TILES_PER_LC + j
                m = mask_pool.tile([P, P], mybir.dt.bfloat16, tag="m")
                nc.vector.tensor_tensor(
                    out=m[:],
                    in0=kidx_f[:, t : t + 1].to_broadcast([P, P]),
                    in1=iotas[lc][:],
                    op=mybir.AluOpType.is_equal,
                )
                rhs = chunks[tile_to_chunk[t]]
                roff = chunk_off[t]
                # hc-outer so each psum group's stop-matmul retires as early
                # as possible on the final tile, letting its copy+store
                # overlap the remaining matmuls
                for hc in range(NHC):
                    for half in range(2):  # hi then lo, same psum accumulator
                        nc.tensor.matmul(
                            ps[hc][:],
                            lhsT=m[:],
                            rhs=rhs[
                                :,
                                roff + half * H + hc * HC : roff + half * H + (hc + 1) * HC,
                            ],
                            start=(j == 0 and half == 0),
                            stop=(j == TILES_PER_LC - 1 and half == 1),
                        )
            o = out_pool.tile([P, H], mybir.dt.float32, tag="o")
            for hc in range(NHC):
                nc.vector.tensor_scalar_mul(o[:, hc * HC : (hc + 1) * HC], ps[hc][:], scale)
                nc.sync.dma_start(
                    out[lc * P : (lc + 1) * P, hc * HC : (hc + 1) * HC],
                    o[:, hc * HC : (hc + 1) * HC],
                )

    nc.compile()
    return nc


def _get_nc():
    global _compiled_nc
    if _compiled_nc is None:
        _compiled_nc = _build_nc()
    return _compiled_nc


def _host_index_math(pos, pad, seq_len, out_len):
    """Exactly mirrors the reference's kernel_idxs computation. Returns
    (kidx [B,S] int64, pooler_mask [B,out_len] bool)."""
    k = int((seq_len // out_len) ** 0.5)
    clamped = np.clip(pos, 0, None).astype(np.int64)
    max_x = clamped[..., 0].max(axis=-1, keepdims=True) + 1  # [B,1]
    kern = clamped // k
    kidx = kern[..., 0] + (max_x // k) * kern[..., 1]  # [B,S]
    B = kidx.shape[0]
    pooler_mask = np.zeros((B, out_len), dtype=bool)
    for b in range(B):
        v = kidx[b]
        v = v[(v >= 0) & (v < out_len)]
        pooler_mask[b, v] = True
    return kidx, pooler_mask


def _numpy_fallback(hs, kidx, pad, out_len):
    hs0 = np.where(pad[..., None], np.float32(0.0), hs)
    B, S_, H_ = hs0.shape
    pooled = np.zeros((B, out_len, H_), dtype=np.float32)
    inv = np.float32(1.0 / (S_ // out_len))
    for b in range(B):
        v = kidx[b]
        ok = (v >= 0) & (v < out_len)
        np.add.at(pooled[b], v[ok], hs0[b, ok] * inv)
    return pooled * np.float32(np.sqrt(np.float64(H_)))


def _prep_core_inputs(hs_b, kidx_dev_b):
    """hs_b [S,H] f32, kidx_dev_b [S] int32 -> {'hsT': [P, NT*W] bf16, 'kidxT': [P, NT] i32}"""
    import ml_dtypes

    bf16 = ml_dtypes.bfloat16
    x = hs_b.reshape(NT, P, H)
    hi = x.astype(bf16)
    lo = (x - hi.astype(np.float32)).astype(bf16)
    cat = np.concatenate([hi, lo], axis=2)  # [NT, P, W]
    hsT_b = np.ascontiguousarray(cat.transpose(1, 0, 2).reshape(P, NT * W))
    kidxT_b = np.ascontiguousarray(kidx_dev_b.reshape(NT, P).T)
    return {"hsT": hsT_b, "kidxT": kidxT_b}


def kernel(hidden_states, pixel_position_ids, padding_positions, output_length):
    hs = np.ascontiguousarray(np.asarray(hidden_states, dtype=np.float32))
    pos = np.asarray(pixel_position_ids)
    pad = np.asarray(padding_positions).astype(bool)
    out_len = int(np.asarray(output_length))

    B, S_, H_ = hs.shape
    kidx, pooler_mask = _host_index_math(pos, pad, S_, out_len)

    # device segment ids: padded rows match no segment (contribute zero)
    kidx_dev = np.where(pad, -1, kidx).astype(np.int32)

    # Fast path requires the fixed problem geometry plus the property that
    # every 128-row tile t only feeds output rows in chunk lc = t // 16.
    fast = B == N_CORES and S_ == S and H_ == H and out_len == L
    if fast:
        lc = (np.arange(S_) // P) // TILES_PER_LC  # [S]
        lo = (lc * P)[None, :]
        fast = bool(np.all((kidx_dev < 0) | ((kidx_dev >= lo) & (kidx_dev < lo + P))))

    if not fast:
        pooled = _numpy_fallback(hs, kidx, pad, out_len)
        return pooled, pooler_mask

    from concourse.bass_utils import run_bass_kernel_spmd

    nc = _get_nc()

    in_maps = [_prep_core_inputs(hs[b], kidx_dev[b]) for b in range(B)]

    res = run_bass_kernel_spmd(nc, in_maps, list(range(N_CORES)), trace=TRACE)

    global LAST_EXEC_NS, LAST_RESULTS
    LAST_EXEC_NS = res.exec_time_ns
    LAST_RESULTS = res

    pooled = np.stack([res.results[b]["out"] for b in range(B)]).astype(np.float32)
    return pooled, pooler_mask


# revision 12
# speedup vs baseline: 1.1286x; 1.1286x over previous
"""Gemma4 vision pooler (position-indexed 4x4 average pool) on 8 TRN2 cores.

Strategy: pure data parallel — batch element b -> core b. On each core the
pooling is a segment reduce over 4096 rows into 256 segments of 16 rows,
done as one-hot matmuls on the tensor engine:

    out[l, h] = sum_s onehot(kidx[s] == l) * hs[s, h],  then * sqrt(H)/16

The host precomputes kidx (the segment id per row, exactly mirroring the
reference index math), splits hs into bf16 hi+lo halves (x ~= hi + lo with
~2^-18 relative error, so the bf16 tensor engine path is near-fp32 exact at
1 cycle/row instead of fp32's 4), pre-transposes to a [128, 32*2304] layout
so every DMA is contiguous per partition, and builds the one-hot masks ON
DEVICE from kidx via iota + is_equal (the 4 MB one-hot never crosses HBM).
"""

import numpy as np

P = 128          # partitions
H = 1152         # hidden size
S = 4096         # sequence length
L = 256          # output length
NT = S // P      # 32 s-tiles of 128 rows
W = 2 * H        # bf16 hi|lo row width per s-tile
TPC = 2          # s-tiles per DMA chunk
NCHUNK = NT // TPC
NHC = 3          # h chunks per matmul group
HC = H // NHC    # 384
N_CORES = 8
TILES_PER_LC = NT // 2  # 16 s-tiles accumulate into each 128-row output chunk

TRACE = False          # set by test harness to capture an NTFF profile
LAST_EXEC_NS = None    # filled when TRACE is set
LAST_RESULTS = None

_compiled_nc = None


def _build_nc():
    from contextlib import ExitStack

    import concourse.bacc as bacc
    import concourse.tile as tile
    from concourse import mybir

    nc = bacc.Bacc("TRN2", target_bir_lowering=False, debug=False)

    # per s-tile t, cols [t*W, t*W+H) = bf16 hi, [t*W+H, (t+1)*W) = bf16 lo
    hsT = nc.dram_tensor("hsT", [P, NT * W], mybir.dt.bfloat16, kind="ExternalInput")
    kidxT = nc.dram_tensor("kidxT", [P, NT], mybir.dt.int32, kind="ExternalInput")
    out = nc.dram_tensor("out", [L, H], mybir.dt.float32, kind="ExternalOutput")

    scale = float(np.float32(np.sqrt(np.float64(H)) / 16.0))

    with ExitStack() as ctx:
        tc = ctx.enter_context(tile.TileContext(nc))
        const_pool = ctx.enter_context(tc.tile_pool(name="const", bufs=1))
        hs_pool = ctx.enter_context(tc.tile_pool(name="hs", bufs=NCHUNK))
        mask_pool = ctx.enter_context(tc.tile_pool(name="mask", bufs=NT))
        out_pool = ctx.enter_context(tc.tile_pool(name="outp", bufs=2))
        psum_pool = ctx.enter_context(tc.tile_pool(name="psum", bufs=1, space="PSUM"))

        kidx_i = const_pool.tile([P, NT], mybir.dt.int32, tag="kidx_i")
        nc.scalar.dma_start(kidx_i[:], kidxT[:])
        kidx_f = const_pool.tile([P, NT], mybir.dt.float32, tag="kidx_f")
        nc.vector.tensor_copy(kidx_f[:], kidx_i[:])

        iotas = []
        for lc in range(2):
            it = const_pool.tile([P, P], mybir.dt.int32, tag=f"iota_i{lc}")
            nc.gpsimd.iota(it[:], pattern=[[1, P]], base=lc * P, channel_multiplier=0)
            itf = const_pool.tile([P, P], mybir.dt.float32, tag=f"iota_f{lc}")
            nc.vector.tensor_copy(itf[:], it[:])
            iotas.append(itf)

        # chunk layout in s-tiles: 15 chunks of 2 tiles + 2 tail chunks of 1
        # tile (shorter final matmul burst after the last DMA lands). All
        # input on the SP HWDGE ring — splitting across both rings measured
        # ~25% slower (packet round-robin between rings).
        chunk_sizes = [2] * 15 + [1, 1]
        assert sum(chunk_sizes) == NT
        tile_to_chunk = {}
        chunk_off = {}
        chunks = []
        t0 = 0
        for c, sz in enumerate(chunk_sizes):
            ch = hs_pool.tile([P, sz * W], mybir.dt.bfloat16, tag="ch", name=f"ch{c}")
            nc.sync.dma_start(ch[:], hsT[:, t0 * W : (t0 + sz) * W])
            chunks.append(ch)
            for j in range(sz):
                tile_to_chunk[t0 + j] = c
                chunk_off[t0 + j] = j * W
            t0 += sz

        for lc in range(2):
            ps = [
                psum_pool.tile([P, HC], mybir.dt.float32, tag=f"ps{lc}_{hc}", name=f"ps{lc}_{hc}")
                for hc in range(NHC)
            ]
            for j in range(TILES_PER_LC):
                t = lc * TILES_PER_LC + j
                m = mask_pool.tile([P, P], mybir.dt.bfloat16, tag="m")
                nc.vector.tensor_tensor(
                    out=m[:],
                    in0=kidx_f[:, t : t + 1].to_broadcast([P, P]),
                    in1=iotas[lc][:],
                    op=mybir.AluOpType.is_equal,
                )
                rhs = chunks[tile_to_chunk[t]]
                roff = chunk_off[t]
                # hc-outer so each psum group's stop-matmul retires as early
                # as possible on the final tile, letting its copy+store
                # overlap the remaining matmuls
                for hc in range(NHC):
                    for half in range(2):  # hi then lo, same psum accumulator
                        nc.tensor.matmul(
                            ps[hc][:],
                            lhsT=m[:],
                            rhs=rhs[
                                :,
                                roff + half * H + hc * HC : roff + half * H + (hc + 1) * HC,
                            ],
                            start=(j == 0 and half == 0),
                            stop=(j == TILES_PER_LC - 1 and half == 1),
                        )
            o = out_pool.tile([P, H], mybir.dt.float32, tag="o")
            for hc in range(NHC):
                nc.vector.tensor_scalar_mul(o[:, hc * HC : (hc + 1) * HC], ps[hc][:], scale)
                nc.scalar.dma_start(
                    out[lc * P : (lc + 1) * P, hc * HC : (hc + 1) * HC],
                    o[:, hc * HC : (hc + 1) * HC],
                )

    nc.compile()
    return nc


def _get_nc():
    global _compiled_nc
    if _compiled_nc is None:
        _compiled_nc = _build_nc()
    return _compiled_nc


def _host_index_math(pos, pad, seq_len, out_len):
    """Exactly mirrors the reference's kernel_idxs computation. Returns
    (kidx [B,S] int64, pooler_mask [B,out_len] bool)."""
    k = int((seq_len // out_len) ** 0.5)
    clamped = np.clip(pos, 0, None).astype(np.int64)
    max_x = clamped[..., 0].max(axis=-1, keepdims=True) + 1  # [B,1]
    kern = clamped // k
    kidx = kern[..., 0] + (max_x // k) * kern[..., 1]  # [B,S]
    B = kidx.shape[0]
    pooler_mask = np.zeros((B, out_len), dtype=bool)
    for b in range(B):
        v = kidx[b]
        v = v[(v >= 0) & (v < out_len)]
        pooler_mask[b, v] = True
    return kidx, pooler_mask


def _numpy_fallback(hs, kidx, pad, out_len):
    hs0 = np.where(pad[..., None], np.float32(0.0), hs)
    B, S_, H_ = hs0.shape
    pooled = np.zeros((B, out_len, H_), dtype=np.float32)
    inv = np.float32(1.0 / (S_ // out_len))
    for b in range(B):
        v = kidx[b]
        ok = (v >= 0) & (v < out_len)
        np.add.at(pooled[b], v[ok], hs0[b, ok] * inv)
    return pooled * np.float32(np.sqrt(np.float64(H_)))


def _prep_core_inputs(hs_b, kidx_dev_b):
    """hs_b [S,H] f32, kidx_dev_b [S] int32 -> {'hsT': [P, NT*W] bf16, 'kidxT': [P, NT] i32}"""
    import ml_dtypes

    bf16 = ml_dtypes.bfloat16
    x = hs_b.reshape(NT, P, H)
    hi = x.astype(bf16)
    lo = (x - hi.astype(np.float32)).astype(bf16)
    cat = np.concatenate([hi, lo], axis=2)  # [NT, P, W]
    hsT_b = np.ascontiguousarray(cat.transpose(1, 0, 2).reshape(P, NT * W))
    kidxT_b = np.ascontiguousarray(kidx_dev_b.reshape(NT, P).T)
    return {"hsT": hsT_b, "kidxT": kidxT_b}


def kernel(hidden_states, pixel_position_ids, padding_positions, output_length):
    hs = np.ascontiguousarray(np.asarray(hidden_states, dtype=np.float32))
    pos = np.asarray(pixel_position_ids)
    pad = np.asarray(padding_positions).astype(bool)
    out_len = int(np.asarray(output_length))

    B, S_, H_ = hs.shape
    kidx, pooler_mask = _host_index_math(pos, pad, S_, out_len)

    # device segment ids: padded rows match no segment (contribute zero)
    kidx_dev = np.where(pad, -1, kidx).astype(np.int32)

    # Fast path requires the fixed problem geometry plus the property that
    # every 128-row tile t only feeds output rows in chunk lc = t // 16.
    fast = B == N_CORES and S_ == S and H_ == H and out_len == L
    if fast:
        lc = (np.arange(S_) // P) // TILES_PER_LC  # [S]
        lo = (lc * P)[None, :]
        fast = bool(np.all((kidx_dev < 0) | ((kidx_dev >= lo) & (kidx_dev < lo + P))))

    if not fast:
        pooled = _numpy_fallback(hs, kidx, pad, out_len)
        return pooled, pooler_mask

    from concourse.bass_utils import run_bass_kernel_spmd

    nc = _get_nc()

    in_maps = [_prep_core_inputs(hs[b], kidx_dev[b]) for b in range(B)]

    res = run_bass_kernel_spmd(nc, in_maps, list(range(N_CORES)), trace=TRACE)

    global LAST_EXEC_NS, LAST_RESULTS
    LAST_EXEC_NS = res.exec_time_ns
    LAST_RESULTS = res

    pooled = np.stack([res.results[b]["out"] for b in range(B)]).astype(np.float32)
    return pooled, pooler_mask
